# revision 24
# baseline (speedup 1.0000x reference)
"""2-layer GCN (GCNConv -> BatchNorm(train) -> ReLU -> GCNConv -> ReLU) on 8 TRN2
NeuronCores, SPMD (one NEFF on all cores).

v10 design (evolved from v3 via NTFF profiles; 2794us -> 1868us):
  - W applied AFTER aggregation (matmul commutes with the scatter-sum):
    L1 gathers raw xs = x*dis rows from a host-shipped node-major table,
    so the per-core h1 table build (52MB HBM + 800 matmuls) is gone.
  - Self-loops are synthetic identity-matmul tiles (lhsT=own rows,
    rhs=identity) seeding each chunk's PSUM accumulator - no gather
    descriptors, no separate self-term passes.
  - dis_src folded into gather-table rows (xs host-side; h2s rows scaled
    during the table build), dis_dst applied per-chunk post-matmul: the
    one-hot is a bare is_eq for BOTH layers (v3 spent ~290us/layer on the
    dissrc multiply, and tensor_tensor with a broadcast operand runs in
    1x DVE mode anyway).
  - Gather calls ~1920 idxs with single_packet=False: SWDGE packets cap at
    ~64 descriptors, so single_packet=True calls beyond 1024 idxs wedge
    the queue (hard device hang); multi-packet big calls amortize the
    ~1us/call fixed cost (was the v3 pacer: GpSimd 65% busy, all in
    per-call SWDGE overhead at 371 calls/layer of <=1024).
  - AllGather split: chunks 0-(SPLIT-1) ship mid-L1 (hidden under the
    gather pass - the mesh AG waits ~70us/MB), header+rest after L1.
    Downstream gates on the LAST collective, so more splits don't help.
  - BN stats ride the second AG's header; L2 self rows are rebuilt from
    the private ag_in copy (no per-core control flow anywhere).
  - AG payload rows are (p k)-interleaved per super so the table-build
    readers see 512B-contiguous runs per partition (halves descriptor
    count vs strict row-major; 256B descs pay a 2x DMA penalty).
  - PSUM pools are bank-granular (8 banks): scoped per phase.

Sharding: nodes padded 100000 -> 102400 = 8*12800, core i owns rows
[i*12800,(i+1)*12800); edges partitioned by dst owner; weights replicated.
"""
import numpy as np
import ml_dtypes

import concourse.bass as bass
import concourse.mybir as mybir
import concourse.tile as tile
from concourse import bacc
from concourse.bass_utils import run_bass_kernel_spmd
from concourse.masks import make_identity

N = 100000
F = 128
NCORES = 8
NPAD = 102400
OWN = NPAD // NCORES          # 12800
CHUNKS = OWN // 128           # 100
SPLIT = 74                    # chunks shipped in the first AG
HROWS = SPLIT * 128           # 9472
GCHUNKS = NPAD // NCORES * NCORES // 128  # 800
NBLK = 4
BLK = NPAD // NBLK            # 25600 (< 32768, int16-addressable)
SEG = OWN + 128               # 12928 rows: c0-69 | header | c70-99
BN_EPS = 1e-5
SC = 2                        # dst chunks per super-chunk
QCAP = 1920                   # max idxs per gather call (121 ring descs;
                              # a call must stay under the 128-desc SWDGE
                              # inflight window or the queue wedges)
BF16 = ml_dtypes.bfloat16

LAST_EXEC_NS = None
LAST_RESULT = None
_cache = {}


def _row_of(c):
    """ag_in row of chunk c's first row (header lives at [HROWS, HROWS+128))."""
    return c * 128 if c < SPLIT else HROWS + 128 + (c - SPLIT) * 128


def _prep(x, edge_index):
    src = np.asarray(edge_index[0]).astype(np.int64)
    dst = np.asarray(edge_index[1]).astype(np.int64)

    deg = np.bincount(dst, minlength=N).astype(np.float32) + 1.0
    dis = np.zeros(NPAD, dtype=np.float32)
    dis[:N] = 1.0 / np.sqrt(deg)

    xs = np.zeros((NPAD, F), dtype=np.float32)
    xs[:N] = np.asarray(x, dtype=np.float32) * dis[:N, None]
    xs_tab = np.ascontiguousarray(xs.astype(BF16))         # [NPAD, F] bf16

    owner = dst // OWN
    chunk = (dst % OWN) // 128
    blk = src // BLK
    cell = ((owner * CHUNKS + chunk) * NBLK + blk).astype(np.int64)
    order = np.lexsort((src, cell))      # ascending src within each cell
    src_s = src[order]
    dst_s = dst[order]

    counts = np.zeros((NCORES, CHUNKS, NBLK), np.int64)
    np.add.at(counts, (owner, chunk, blk), 1)
    C = counts.max(axis=0)
    C = ((C + 127) // 128) * 128         # zero cells stay zero

    starts = np.zeros(NCORES * CHUNKS * NBLK + 1, dtype=np.int64)
    starts[1:] = np.cumsum(counts.reshape(-1))

    # super-chunk slot layout: for each super s: for each block b: the SC
    # cells (c, b) back to back.  Gather call = one (s, b) segment, split
    # to <=QCAP idxs (balanced so no tiny remainder call).
    nsup = CHUNKS // SC
    slot_pos = {}
    sup_meta = []
    off = 0
    for s in range(nsup):
        chs = list(range(s * SC, (s + 1) * SC))
        sup_off = off
        seg_calls = []
        for b in range(NBLK):
            call_off = off
            for c in chs:
                slot_pos[(c, b)] = off
                off += int(C[c, b])
            seg_n = off - call_off
            if seg_n:
                k = -(-seg_n // QCAP)            # calls for this segment
                per = ((seg_n // k) // 128) * 128
                sub = 0
                for ki in range(k):
                    n = per if ki < k - 1 else seg_n - per * (k - 1)
                    assert 0 < n <= 2032, n   # 128-desc inflight window
                    seg_calls.append((b, call_off + sub, n))
                    sub += n
        chunk_of = []
        for b in range(NBLK):
            for ci, c in enumerate(chs):
                chunk_of.extend([ci] * (int(C[c, b]) // 128))
        last = {}
        for t, ci in enumerate(chunk_of):
            last[ci] = t
        sup_meta.append({"off": sup_off, "ntiles": len(chunk_of),
                         "chunk_of": chunk_of, "last": last,
                         "calls": seg_calls, "chunks": chs})
    tot = off
    ntiles = tot // 128

    per_core = []
    for i in range(NCORES):
        srcidx = np.zeros(tot, dtype=np.int16)                # pads hit row 0
        dstloc = np.full(tot, -1.0, dtype=np.float32)         # pads no column
        for c in range(CHUNKS):
            for b in range(NBLK):
                k = (i * CHUNKS + c) * NBLK + b
                m = int(counts[i, c, b])
                if m:
                    o = slot_pos[(c, b)]
                    sl = slice(starts[k], starts[k] + m)
                    srcidx[o:o + m] = (src_s[sl] - b * BLK).astype(np.int16)
                    dstloc[o:o + m] = (dst_s[sl] % 128).astype(np.float32)
        iw = srcidx.reshape(tot // 16, 16).T                  # [16, tot/16]
        srcidx_w = np.ascontiguousarray(np.tile(iw, (8, 1)))  # [128, tot/16]
        dstloc_t = np.ascontiguousarray(
            dstloc.reshape(ntiles, 128).T.astype(BF16))
        disT = np.ascontiguousarray(
            dis[i * OWN:(i + 1) * OWN].reshape(CHUNKS, 128).T)
        xs_own = np.ascontiguousarray(xs_tab[i * OWN:(i + 1) * OWN])
        per_core.append({"srcidx": srcidx_w, "dstloc": dstloc_t,
                         "disT": disT, "xs_own": xs_own})

    disG = np.ascontiguousarray(dis.reshape(GCHUNKS, 128).T)  # [128, 800]

    consts = {"tot": tot, "ntiles": ntiles, "sup_meta": sup_meta}
    return consts, xs_tab, disG, per_core


def _build(consts):
    tot = consts["tot"]
    ntiles = consts["ntiles"]
    sup_meta = consts["sup_meta"]

    f32 = mybir.dt.float32
    bf16 = mybir.dt.bfloat16
    AF = mybir.ActivationFunctionType
    OP = mybir.AluOpType
    nc = bacc.Bacc("TRN2", target_bir_lowering=False, debug=False,
                   num_devices=NCORES, num_swdge_queues=4,
                   dynamic_dma_scratch_size=32768)

    xstab_d = nc.dram_tensor("xs_tab", [NPAD, F], bf16, kind="ExternalInput").ap()
    xsown_d = nc.dram_tensor("xs_own", [OWN, F], bf16, kind="ExternalInput").ap()
    W1_d = nc.dram_tensor("W1b", [F, F], bf16, kind="ExternalInput").ap()
    W2_d = nc.dram_tensor("W2b", [F, F], bf16, kind="ExternalInput").ap()
    gamma_d = nc.dram_tensor("gamma_c", [F, 1], f32, kind="ExternalInput").ap()
    beta_d = nc.dram_tensor("beta_c", [F, 1], f32, kind="ExternalInput").ap()
    b2m_d = nc.dram_tensor("b2_mat", [128, F], f32, kind="ExternalInput").ap()
    disT_d = nc.dram_tensor("disT", [128, CHUNKS], f32, kind="ExternalInput").ap()
    disG_d = nc.dram_tensor("disG", [128, GCHUNKS], f32, kind="ExternalInput").ap()
    srcidx_d = nc.dram_tensor("srcidx", [128, tot // 16], mybir.dt.int16,
                              kind="ExternalInput").ap()
    dstloc_d = nc.dram_tensor("dstloc", [128, ntiles], bf16,
                              kind="ExternalInput").ap()
    out_d = nc.dram_tensor("out", [OWN, F], f32, kind="ExternalOutput").ap()

    h2s = nc.dram_tensor("h2s_tab", [NPAD, F], bf16)
    ag_in = nc.dram_tensor("ag_in", [SEG, F], bf16)
    ag_outA = nc.dram_tensor("ag_outA", [NCORES * HROWS, F], bf16,
                             addr_space="Shared")
    SEGB = SEG - HROWS            # 3968 rows: header + chunks 70-99
    ag_outB = nc.dram_tensor("ag_outB", [NCORES * SEGB, F], bf16,
                             addr_space="Shared")

    with tile.TileContext(nc) as tc:
        with tc.tile_pool(name="const", bufs=1) as constp, \
             tc.tile_pool(name="big", bufs=1) as bigp, \
             tc.tile_pool(name="h", bufs=5) as hp, \
             tc.tile_pool(name="gbuf", bufs=4) as gbufp, \
             tc.tile_pool(name="oh", bufs=6) as ohp, \
             tc.tile_pool(name="wk", bufs=4) as wp, \
             tc.tile_pool(name="st", bufs=1) as stp:

            # ---- constants ----
            W1_t = constp.tile([F, F], bf16)
            W2_t = constp.tile([F, F], bf16)
            ident_f = constp.tile([128, 128], f32)
            ident_b = constp.tile([128, 128], bf16)
            iota4 = constp.tile([128, 4, 128], bf16)
            ones_f = constp.tile([128, 1], f32)
            ones_b = constp.tile([128, 1], bf16)
            gamma_t = constp.tile([F, 1], f32)
            beta_t = constp.tile([F, 1], f32)
            b2m_t = constp.tile([128, F], f32)
            disT_t = constp.tile([128, CHUNKS], f32)
            disG_t = constp.tile([128, GCHUNKS], f32)
            nc.sync.dma_start(out=W1_t[:], in_=W1_d[:])
            nc.sync.dma_start(out=W2_t[:], in_=W2_d[:])
            nc.sync.dma_start(out=gamma_t[:], in_=gamma_d[:])
            nc.sync.dma_start(out=beta_t[:], in_=beta_d[:])
            nc.sync.dma_start(out=b2m_t[:], in_=b2m_d[:])
            nc.sync.dma_start(out=disT_t[:], in_=disT_d[:])
            nc.sync.dma_start(out=disG_t[:], in_=disG_d[:])
            make_identity(nc, ident_f[:])
            make_identity(nc, ident_b[:])
            iota_i = constp.tile([128, 128], mybir.dt.int32)
            nc.gpsimd.iota(iota_i[:], pattern=[[1, 128]], base=0,
                           channel_multiplier=0)
            for k in range(4):
                nc.vector.tensor_copy(out=iota4[:, k, :], in_=iota_i[:])
            nc.vector.memset(ones_f[:], 1.0)
            nc.vector.memset(ones_b[:], 1.0)

            srcidx_sb = bigp.tile([128, tot // 16], mybir.dt.int16)
            dstloc_sb = bigp.tile([128, ntiles], bf16)
            xsown_sb = bigp.tile([128, CHUNKS, 128], bf16)
            aown_sb = bigp.tile([128, CHUNKS, 128], bf16)
            qtr = (tot // 16) // 4
            for kq in range(4):
                lo = kq * qtr
                hi = (kq + 1) * qtr if kq < 3 else tot // 16
                nc.sync.dma_start(out=srcidx_sb[:, lo:hi],
                                  in_=srcidx_d[:, lo:hi])
            nc.sync.dma_start(out=dstloc_sb[:], in_=dstloc_d[:])
            nc.sync.dma_start(
                out=xsown_sb[:],
                in_=xsown_d.rearrange("(k p) f -> p k f", p=128))

            # ---- shared gather/scatter pass (super-chunk granularity) ----
            # acc_c[f, d] = sum_e table[src_e, f] * onehot[e, d] + own[d, f]
            def layer_pass(table_ap, own_sb, psS, super_stage, hooks=None):
                qn = 0
                for si, sm in enumerate(sup_meta):
                    TS = sm["ntiles"]
                    gb = gbufp.tile([128, max(TS, 1), 128], bf16, tag="gb")
                    base_t = sm["off"] // 128
                    for (b, coff, n) in sm["calls"]:
                        ol = coff - sm["off"]
                        nc.gpsimd.dma_gather(
                            gb[:, ol // 128:(ol + n + 127) // 128, :],
                            table_ap[b * BLK:(b + 1) * BLK, :],
                            srcidx_sb[:, coff // 16:(coff + n) // 16],
                            n, n, F, queue_num=qn,
                            single_packet=(n <= 1024))
                        qn = (qn + 1) % 4
                    accs = [psS.tile([128, F], f32, tag="acc",
                                     name=f"acc{k}")
                            for k in range(SC)]
                    # self-loop seed: acc_c = own_rows_c^T (identity one-hot)
                    for ci, c in enumerate(sm["chunks"]):
                        nc.tensor.matmul(out=accs[ci][:],
                                         lhsT=own_sb[:, c, :],
                                         rhs=ident_b[:],
                                         start=True,
                                         stop=(ci not in sm["last"]))
                    t = 0
                    while t < TS:
                        w = min(4, TS - t)
                        oh = ohp.tile([128, 4, 128], bf16, tag="oh")
                        nc.vector.tensor_tensor(
                            out=oh[:, :w, :],
                            in0=dstloc_sb[:, base_t + t:base_t + t + w]
                                .to_broadcast([128, w, 128]),
                            in1=iota4[:, :w, :], op=OP.is_equal)
                        for j in range(w):
                            ci = sm["chunk_of"][t + j]
                            nc.tensor.matmul(out=accs[ci][:],
                                             lhsT=gb[:, t + j, :],
                                             rhs=oh[:, j, :],
                                             start=False,
                                             stop=(sm["last"][ci] == t + j))
                        t += w
                    super_stage(sm, accs)
                    if hooks and si in hooks:
                        hooks[si]()

            # ================= L1 pass (scoped PSUM pools) =================
            with tc.tile_pool(name="psS1", bufs=SC, space="PSUM") as psS1, \
                 tc.tile_pool(name="psW1", bufs=2, space="PSUM") as psW1, \
                 tc.tile_pool(name="psT1", bufs=2, space="PSUM") as psT1, \
                 tc.tile_pool(name="pss", bufs=1, space="PSUM") as pss:

                # BN stat accumulators (separate banks)
                sum_ps = pss.tile([128, 1], f32, name="sum_ps")
                sq_ps = pss.tile([128, 1], f32, name="sq_ps")

                # ---- L1: acc -> @W1 -> *dis_dst -> stats + transp. ship ----
                def l1_stage(sm, accs):
                    trp = psT1.tile([128, SC, 128], f32, tag="a")
                    for ci, c in enumerate(sm["chunks"]):
                        tc_sb = wp.tile([128, 128], bf16, tag="tc")
                        nc.scalar.activation(tc_sb[:], accs[ci][:], AF.Copy)
                        ps2 = psW1.tile([128, 128], f32, tag="w")
                        nc.tensor.matmul(out=ps2[:], lhsT=tc_sb[:],
                                         rhs=W1_t[:], start=True, stop=True)
                        asb = wp.tile([128, 128], f32, tag="asb")
                        nc.vector.tensor_scalar_mul(out=asb[:], in0=ps2[:],
                                                    scalar1=disT_t[:, c:c + 1])
                        nc.tensor.matmul(out=sum_ps[:], lhsT=asb[:],
                                         rhs=ones_f[:],
                                         start=(c == 0), stop=(c == CHUNKS - 1))
                        sq = wp.tile([128, 128], bf16, tag="sq")
                        nc.scalar.square(sq[:], asb[:])
                        nc.tensor.matmul(out=sq_ps[:], lhsT=sq[:],
                                         rhs=ones_b[:],
                                         start=(c == 0), stop=(c == CHUNKS - 1))
                        nc.tensor.transpose(out=trp[:, ci, :], in_=asb[:],
                                            identity=ident_f[:])
                    tst = wp.tile([128, SC, 128], bf16, tag="tT")
                    nc.scalar.activation(tst[:], trp[:], AF.Copy)
                    r0 = _row_of(sm["chunks"][0])
                    nc.sync.dma_start(
                        out=ag_in[r0:r0 + SC * 128, :]
                            .rearrange("(p k) f -> p k f", p=128),
                        in_=tst[:])

                def ship_first_half():
                    nc.gpsimd.collective_compute(
                        "AllGather", OP.bypass, ins=[ag_in.ap()[0:HROWS, :]],
                        outs=[ag_outA.ap()],
                        replica_groups=[list(range(NCORES))])

                layer_pass(xstab_d, xsown_sb, psS1, l1_stage,
                           hooks={SPLIT // SC - 1: ship_first_half})

                # ---- stats header -> ag_in rows [6400,6528) (rows 0,1) ----
                stats2 = stp.tile([128, 2], f32)
                nc.vector.tensor_copy(out=stats2[:, 0:1], in_=sum_ps[:])
                nc.vector.tensor_copy(out=stats2[:, 1:2], in_=sq_ps[:])
                stpad = stp.tile([128, 128], f32)
                nc.vector.memset(stpad[:], 0.0)
                nc.vector.tensor_copy(out=stpad[:, 0:2], in_=stats2[:])
                trs = psT1.tile([128, SC, 128], f32, tag="a")
                nc.tensor.transpose(out=trs[:, 0, :], in_=stpad[:],
                                    identity=ident_f[:])
                stag = stp.tile([128, 128], bf16)
                nc.scalar.activation(stag[:], trs[:, 0, :], AF.Copy)
                nc.sync.dma_start(out=ag_in[HROWS:HROWS + 128, :], in_=stag[:])

            nc.gpsimd.collective_compute(
                "AllGather", OP.bypass, ins=[ag_in.ap()[HROWS:SEG, :]],
                outs=[ag_outB.ap()],
                replica_groups=[list(range(NCORES))])

            # ================= mid phase (scoped PSUM pools) ===============
            with tc.tile_pool(name="psG", bufs=1, space="PSUM") as psG, \
                 tc.tile_pool(name="psAB", bufs=4, space="PSUM") as psAB:

                # ---- global BN stats from the 8 headers ----
                gst = stp.tile([16, 128], bf16)
                agv = ag_outB.ap().rearrange("(i s) f -> i s f", i=NCORES)
                nc.sync.dma_start(out=gst[0:8, :], in_=agv[:, 0, :])
                nc.sync.dma_start(out=gst[8:16, :], in_=agv[:, 1, :])
                gpad = stp.tile([128, 128], f32)
                nc.vector.memset(gpad[:], 0.0)
                nc.vector.tensor_copy(out=gpad[0:16, :], in_=gst[:])
                gtr = psG.tile([128, 128], f32)
                nc.tensor.transpose(out=gtr[:], in_=gpad[:],
                                    identity=ident_f[:])
                # cols 0..7 = per-core sums, 8..15 = per-core sumsqs
                gred = stp.tile([128, 16], f32)
                nc.vector.tensor_copy(out=gred[:], in_=gtr[:, 0:16])
                nc.vector.tensor_tensor(out=gred[:, 0:4], in0=gred[:, 0:4],
                                        in1=gred[:, 4:8], op=OP.add)
                nc.vector.tensor_tensor(out=gred[:, 8:12], in0=gred[:, 8:12],
                                        in1=gred[:, 12:16], op=OP.add)
                nc.vector.tensor_tensor(out=gred[:, 0:2], in0=gred[:, 0:2],
                                        in1=gred[:, 2:4], op=OP.add)
                nc.vector.tensor_tensor(out=gred[:, 8:10], in0=gred[:, 8:10],
                                        in1=gred[:, 10:12], op=OP.add)
                nc.vector.tensor_tensor(out=gred[:, 0:1], in0=gred[:, 0:1],
                                        in1=gred[:, 1:2], op=OP.add)
                nc.vector.tensor_tensor(out=gred[:, 8:9], in0=gred[:, 8:9],
                                        in1=gred[:, 9:10], op=OP.add)

                mean_t = stp.tile([128, 1], f32)
                ex2_t = stp.tile([128, 1], f32)
                var_t = stp.tile([128, 1], f32)
                sd_t = stp.tile([128, 1], f32)
                rstd_t = stp.tile([128, 1], f32)
                scale_c = stp.tile([128, 1], f32)
                shift_c = stp.tile([128, 1], f32)
                eps_t = stp.tile([128, 1], f32)
                nc.vector.tensor_scalar_mul(out=mean_t[:], in0=gred[:, 0:1],
                                            scalar1=1.0 / N)
                nc.vector.tensor_scalar_mul(out=ex2_t[:], in0=gred[:, 8:9],
                                            scalar1=1.0 / N)
                nc.vector.tensor_tensor(out=var_t[:], in0=mean_t[:],
                                        in1=mean_t[:], op=OP.mult)
                nc.vector.tensor_tensor(out=var_t[:], in0=ex2_t[:],
                                        in1=var_t[:], op=OP.subtract)
                nc.vector.tensor_scalar_max(out=var_t[:], in0=var_t[:],
                                            scalar1=0.0)
                nc.vector.memset(eps_t[:], BN_EPS)
                nc.scalar.activation(sd_t[:], var_t[:], AF.Sqrt, bias=eps_t[:])
                nc.vector.reciprocal(out=rstd_t[:], in_=sd_t[:])
                nc.vector.tensor_tensor(out=scale_c[:], in0=rstd_t[:],
                                        in1=gamma_t[:], op=OP.mult)
                nc.vector.tensor_tensor(out=shift_c[:], in0=mean_t[:],
                                        in1=scale_c[:], op=OP.mult)
                nc.vector.tensor_tensor(out=shift_c[:], in0=beta_t[:],
                                        in1=shift_c[:], op=OP.subtract)

                # activate feature-major groups -> transpose -> *dis -> store
                def act_group(src_ap, dst_sb, dst_c0, c0, w, dis_t, goff):
                    stb = hp.tile([128, w, 128], bf16, tag=f"cb{w}")
                    if w == 2:
                        nc.sync.dma_start(
                            out=stb[:],
                            in_=src_ap.rearrange("(p k) f -> p k f", p=128))
                    else:
                        nc.sync.dma_start(
                            out=stb[:].rearrange("p (s k) f -> p s k f",
                                                 s=w // 2, k=2),
                            in_=src_ap.rearrange("(s p k) f -> p s k f",
                                                 s=w // 2, p=128))
                    h2a = wp.tile([128, w, 128], f32, tag=f"h2{w}")
                    nc.scalar.activation(h2a[:], stb[:], AF.Relu,
                                         bias=shift_c[:], scale=scale_c[:])
                    trp = psAB.tile([128, 4, 128], f32, tag="ab")
                    for j in range(w):
                        nc.tensor.transpose(out=trp[:, j, :], in_=h2a[:, j, :],
                                            identity=ident_f[:])
                    for j in range(w):
                        c = c0 + j
                        if c % 3 == 0:
                            nc.scalar.activation(
                                dst_sb[:, dst_c0 + j, :], trp[:, j, :],
                                AF.Copy,
                                scale=dis_t[:, goff + c:goff + c + 1])
                        else:
                            nc.vector.tensor_scalar_mul(
                                out=dst_sb[:, dst_c0 + j, :], in0=trp[:, j, :],
                                scalar1=dis_t[:, goff + c:goff + c + 1])

                RUNS = [(0, SPLIT), (SPLIT, CHUNKS - SPLIT)]

                def half_groups():
                    for base, ln in RUNS:
                        c0 = base
                        while c0 < base + ln:
                            w = min(4, base + ln - c0)
                            yield c0, w
                            c0 += w

                # ---- L2 self rows: a_own*dis from the private ag_in copy ----
                for c0, w in half_groups():
                    r0 = _row_of(c0)
                    act_group(ag_in.ap()[r0:r0 + w * 128, :],
                              aown_sb, c0, c0, w, disT_t, 0)

                # ---- h2s table: relu(bn(agg1))*dis, node-major, all nodes ----
                for i in range(NCORES):
                    for c0, w in half_groups():
                        if c0 < SPLIT:
                            r = i * HROWS + c0 * 128
                            src = ag_outA.ap()[r:r + w * 128, :]
                        else:
                            r = i * SEGB + 128 + (c0 - SPLIT) * 128
                            src = ag_outB.ap()[r:r + w * 128, :]
                        hb2 = hp.tile([128, w, F], bf16, tag=f"hh{w}")
                        act_group(src, hb2, 0, c0, w, disG_t, i * CHUNKS)
                        orow = i * OWN + c0 * 128
                        nc.sync.dma_start(
                            out=h2s[orow:orow + w * 128, :]
                                .rearrange("(k p) f -> p k f", p=128),
                            in_=hb2[:])

            # ================= L2 pass (scoped PSUM pools) =================
            with tc.tile_pool(name="psS2", bufs=SC, space="PSUM") as psS2, \
                 tc.tile_pool(name="psW2", bufs=2, space="PSUM") as psW2:

                # ---- L2: acc -> @W2 -> *dis_dst -> +b2 -> relu -> out ----
                def l2_stage(sm, accs):
                    o1 = wp.tile([128, SC, 128], f32, tag="o1")
                    for ci, c in enumerate(sm["chunks"]):
                        tc_sb = wp.tile([128, 128], bf16, tag="tc")
                        nc.scalar.activation(tc_sb[:], accs[ci][:], AF.Copy)
                        ps2 = psW2.tile([128, 128], f32, tag="w")
                        nc.tensor.matmul(out=ps2[:], lhsT=tc_sb[:],
                                         rhs=W2_t[:], start=True, stop=True)
                        asb = wp.tile([128, 128], f32, tag="asb")
                        nc.vector.tensor_scalar_mul(out=asb[:], in0=ps2[:],
                                                    scalar1=disT_t[:, c:c + 1])
                        nc.vector.tensor_tensor(out=o1[:, ci, :], in0=asb[:],
                                                in1=b2m_t[:], op=OP.add)
                    ot = wp.tile([128, SC, 128], f32, tag="ot")
                    nc.scalar.activation(ot[:], o1[:], AF.Relu)
                    r0 = sm["chunks"][0] * 128
                    nc.sync.dma_start(
                        out=out_d[r0:r0 + SC * 128, :]
                            .rearrange("(k p) f -> p k f", p=128),
                        in_=ot[:])

                layer_pass(h2s.ap(), aown_sb, psS2, l2_stage)

    nc.compile()
    return nc


def kernel(**inputs):
    global LAST_EXEC_NS, LAST_RESULT
    import os
    x = inputs["x"]
    W1 = np.asarray(inputs["W1"], dtype=np.float32)
    W2 = np.asarray(inputs["W2"], dtype=np.float32)
    gamma = np.asarray(inputs["gamma"], dtype=np.float32)
    beta = np.asarray(inputs["beta"], dtype=np.float32)
    b2 = np.asarray(inputs["b2"], dtype=np.float32)
    edge_index = inputs["edge_index"]

    key = (hash(np.asarray(edge_index)[:, ::997].tobytes()),)
    if key not in _cache:
        consts, xs_tab, disG, per_core = _prep(x, edge_index)
        nc = _build(consts)
        _cache[key] = (consts, nc)
    else:
        consts, nc = _cache[key]
        _, xs_tab, disG, per_core = _prep(x, edge_index)

    shared = {
        "xs_tab": xs_tab,
        "disG": disG,
        "W1b": W1.astype(BF16), "W2b": W2.astype(BF16),
        "gamma_c": gamma.reshape(F, 1).copy(),
        "beta_c": beta.reshape(F, 1).copy(),
        "b2_mat": np.ascontiguousarray(np.broadcast_to(b2.reshape(1, F),
                                                       (128, F))),
    }
    in_maps = []
    for i in range(NCORES):
        m = dict(shared)
        m.update(per_core[i])
        in_maps.append(m)

    trace = bool(os.environ.get("BASS_GCN_TRACE"))
    res = run_bass_kernel_spmd(nc, in_maps, list(range(NCORES)), trace=trace)
    LAST_EXEC_NS = res.exec_time_ns
    LAST_RESULT = res

    out = np.concatenate([res.results[i]["out"] for i in range(NCORES)], axis=0)
    return np.ascontiguousarray(out[:N]).astype(np.float32)


# revision 26
# speedup vs baseline: 1.0460x; 1.0460x over previous
"""2-layer GCN (GCNConv -> BatchNorm(train) -> ReLU -> GCNConv -> ReLU) on 8 TRN2
NeuronCores, SPMD (one NEFF on all cores).

v10 design (evolved from v3 via NTFF profiles; 2794us -> 1868us):
  - W applied AFTER aggregation (matmul commutes with the scatter-sum):
    L1 gathers raw xs = x*dis rows from a host-shipped node-major table,
    so the per-core h1 table build (52MB HBM + 800 matmuls) is gone.
  - Self-loops are synthetic identity-matmul tiles (lhsT=own rows,
    rhs=identity) seeding each chunk's PSUM accumulator - no gather
    descriptors, no separate self-term passes.
  - dis_src folded into gather-table rows (xs host-side; h2s rows scaled
    during the table build), dis_dst applied per-chunk post-matmul: the
    one-hot is a bare is_eq for BOTH layers (v3 spent ~290us/layer on the
    dissrc multiply, and tensor_tensor with a broadcast operand runs in
    1x DVE mode anyway).
  - Gather calls ~1920 idxs with single_packet=False: SWDGE packets cap at
    ~64 descriptors, so single_packet=True calls beyond 1024 idxs wedge
    the queue (hard device hang); multi-packet big calls amortize the
    ~1us/call fixed cost (was the v3 pacer: GpSimd 65% busy, all in
    per-call SWDGE overhead at 371 calls/layer of <=1024).
  - AllGather split: chunks 0-(SPLIT-1) ship mid-L1 (hidden under the
    gather pass - the mesh AG waits ~70us/MB), header+rest after L1.
    Downstream gates on the LAST collective, so more splits don't help.
  - BN stats ride the second AG's header; L2 self rows are rebuilt from
    the private ag_in copy (no per-core control flow anywhere).
  - AG payload rows are (p k)-interleaved per super so the table-build
    readers see 512B-contiguous runs per partition (halves descriptor
    count vs strict row-major; 256B descs pay a 2x DMA penalty).
  - PSUM pools are bank-granular (8 banks): scoped per phase.

Sharding: nodes padded 100000 -> 102400 = 8*12800, core i owns rows
[i*12800,(i+1)*12800); edges partitioned by dst owner; weights replicated.
"""
import numpy as np
import ml_dtypes

import concourse.bass as bass
import concourse.mybir as mybir
import concourse.tile as tile
from concourse import bacc
from concourse.bass_utils import run_bass_kernel_spmd
from concourse.masks import make_identity

N = 100000
F = 128
NCORES = 8
NPAD = 102400
OWN = NPAD // NCORES          # 12800
CHUNKS = OWN // 128           # 100
SPLIT = 74                    # chunks shipped in the first AG
HROWS = SPLIT * 128           # 9472
GCHUNKS = NPAD // NCORES * NCORES // 128  # 800
NBLK = 4
BLK = NPAD // NBLK            # 25600 (< 32768, int16-addressable)
SEG = OWN + 128               # 12928 rows: c0-69 | header | c70-99
BN_EPS = 1e-5
SC = 2                        # dst chunks per super-chunk
QCAP = 1920                   # max idxs per gather call (121 ring descs;
                              # a call must stay under the 128-desc SWDGE
                              # inflight window or the queue wedges)
BF16 = ml_dtypes.bfloat16

LAST_EXEC_NS = None
LAST_RESULT = None
_cache = {}


def _row_of(c):
    """ag_in row of chunk c's first row (header lives at [HROWS, HROWS+128))."""
    return c * 128 if c < SPLIT else HROWS + 128 + (c - SPLIT) * 128


def _prep(x, edge_index):
    src = np.asarray(edge_index[0]).astype(np.int64)
    dst = np.asarray(edge_index[1]).astype(np.int64)

    deg = np.bincount(dst, minlength=N).astype(np.float32) + 1.0
    dis = np.zeros(NPAD, dtype=np.float32)
    dis[:N] = 1.0 / np.sqrt(deg)

    xs = np.zeros((NPAD, F), dtype=np.float32)
    xs[:N] = np.asarray(x, dtype=np.float32) * dis[:N, None]
    xs_tab = np.ascontiguousarray(xs.astype(BF16))         # [NPAD, F] bf16

    owner = dst // OWN
    chunk = (dst % OWN) // 128
    blk = src // BLK
    cell = ((owner * CHUNKS + chunk) * NBLK + blk).astype(np.int64)
    order = np.lexsort((src, cell))      # ascending src within each cell
    src_s = src[order]
    dst_s = dst[order]

    counts = np.zeros((NCORES, CHUNKS, NBLK), np.int64)
    np.add.at(counts, (owner, chunk, blk), 1)
    C = counts.max(axis=0)
    C = ((C + 127) // 128) * 128         # zero cells stay zero

    starts = np.zeros(NCORES * CHUNKS * NBLK + 1, dtype=np.int64)
    starts[1:] = np.cumsum(counts.reshape(-1))

    # super-chunk slot layout: for each super s: for each block b: the SC
    # cells (c, b) back to back.  Gather call = one (s, b) segment, split
    # to <=QCAP idxs (balanced so no tiny remainder call).
    nsup = CHUNKS // SC
    slot_pos = {}
    sup_meta = []
    off = 0
    for s in range(nsup):
        chs = list(range(s * SC, (s + 1) * SC))
        sup_off = off
        seg_calls = []
        for b in range(NBLK):
            call_off = off
            for c in chs:
                slot_pos[(c, b)] = off
                off += int(C[c, b])
            seg_n = off - call_off
            if seg_n:
                k = -(-seg_n // QCAP)            # calls for this segment
                per = ((seg_n // k) // 128) * 128
                sub = 0
                for ki in range(k):
                    n = per if ki < k - 1 else seg_n - per * (k - 1)
                    assert 0 < n <= 2032, n   # 128-desc inflight window
                    seg_calls.append((b, call_off + sub, n))
                    sub += n
        chunk_of = []
        for b in range(NBLK):
            for ci, c in enumerate(chs):
                chunk_of.extend([ci] * (int(C[c, b]) // 128))
        last = {}
        for t, ci in enumerate(chunk_of):
            last[ci] = t
        sup_meta.append({"off": sup_off, "ntiles": len(chunk_of),
                         "chunk_of": chunk_of, "last": last,
                         "calls": seg_calls, "chunks": chs})
    tot = off
    ntiles = tot // 128

    per_core = []
    for i in range(NCORES):
        srcidx = np.zeros(tot, dtype=np.int16)                # pads hit row 0
        dstloc = np.full(tot, -1.0, dtype=np.float32)         # pads no column
        for c in range(CHUNKS):
            for b in range(NBLK):
                k = (i * CHUNKS + c) * NBLK + b
                m = int(counts[i, c, b])
                if m:
                    o = slot_pos[(c, b)]
                    sl = slice(starts[k], starts[k] + m)
                    srcidx[o:o + m] = (src_s[sl] - b * BLK).astype(np.int16)
                    dstloc[o:o + m] = (dst_s[sl] % 128).astype(np.float32)
        iw = srcidx.reshape(tot // 16, 16).T                  # [16, tot/16]
        srcidx_w = np.ascontiguousarray(np.tile(iw, (8, 1)))  # [128, tot/16]
        dstloc_t = np.ascontiguousarray(
            dstloc.reshape(ntiles, 128).T.astype(BF16))
        disT = np.ascontiguousarray(
            dis[i * OWN:(i + 1) * OWN].reshape(CHUNKS, 128).T)
        xs_own = np.ascontiguousarray(xs_tab[i * OWN:(i + 1) * OWN])
        per_core.append({"srcidx": srcidx_w, "dstloc": dstloc_t,
                         "disT": disT, "xs_own": xs_own})

    disG = np.ascontiguousarray(dis.reshape(GCHUNKS, 128).T)  # [128, 800]

    consts = {"tot": tot, "ntiles": ntiles, "sup_meta": sup_meta}
    return consts, xs_tab, disG, per_core


def _build(consts):
    tot = consts["tot"]
    ntiles = consts["ntiles"]
    sup_meta = consts["sup_meta"]

    f32 = mybir.dt.float32
    bf16 = mybir.dt.bfloat16
    AF = mybir.ActivationFunctionType
    OP = mybir.AluOpType
    nc = bacc.Bacc("TRN2", target_bir_lowering=False, debug=False,
                   num_devices=NCORES, num_swdge_queues=4,
                   dynamic_dma_scratch_size=32768)

    xstab_d = nc.dram_tensor("xs_tab", [NPAD, F], bf16, kind="ExternalInput").ap()
    xsown_d = nc.dram_tensor("xs_own", [OWN, F], bf16, kind="ExternalInput").ap()
    W1_d = nc.dram_tensor("W1b", [F, F], bf16, kind="ExternalInput").ap()
    W2_d = nc.dram_tensor("W2b", [F, F], bf16, kind="ExternalInput").ap()
    gamma_d = nc.dram_tensor("gamma_c", [F, 1], f32, kind="ExternalInput").ap()
    beta_d = nc.dram_tensor("beta_c", [F, 1], f32, kind="ExternalInput").ap()
    b2m_d = nc.dram_tensor("b2_mat", [128, F], f32, kind="ExternalInput").ap()
    disT_d = nc.dram_tensor("disT", [128, CHUNKS], f32, kind="ExternalInput").ap()
    disG_d = nc.dram_tensor("disG", [128, GCHUNKS], f32, kind="ExternalInput").ap()
    srcidx_d = nc.dram_tensor("srcidx", [128, tot // 16], mybir.dt.int16,
                              kind="ExternalInput").ap()
    dstloc_d = nc.dram_tensor("dstloc", [128, ntiles], bf16,
                              kind="ExternalInput").ap()
    out_d = nc.dram_tensor("out", [OWN, F], f32, kind="ExternalOutput").ap()

    h2s = nc.dram_tensor("h2s_tab", [NPAD, F], bf16)
    ag_in = nc.dram_tensor("ag_in", [SEG, F], bf16)
    ag_outA = nc.dram_tensor("ag_outA", [NCORES * HROWS, F], bf16,
                             addr_space="Shared")
    SEGB = SEG - HROWS            # 3968 rows: header + chunks 70-99
    ag_outB = nc.dram_tensor("ag_outB", [NCORES * SEGB, F], bf16,
                             addr_space="Shared")

    with tile.TileContext(nc) as tc:
        with tc.tile_pool(name="const", bufs=1) as constp, \
             tc.tile_pool(name="big", bufs=1) as bigp, \
             tc.tile_pool(name="h", bufs=3) as hp, \
             tc.tile_pool(name="gbuf", bufs=4) as gbufp, \
             tc.tile_pool(name="oh", bufs=6) as ohp, \
             tc.tile_pool(name="wk", bufs=4) as wp, \
             tc.tile_pool(name="st", bufs=1) as stp:

            # ---- constants ----
            W1_t = constp.tile([F, F], bf16)
            W2_t = constp.tile([F, F], bf16)
            ident_f = constp.tile([128, 128], f32)
            ident_b = constp.tile([128, 128], bf16)
            iota4 = constp.tile([128, 4, 128], bf16)
            ones_f = constp.tile([128, 1], f32)
            ones_b = constp.tile([128, 1], bf16)
            gamma_t = constp.tile([F, 1], f32)
            beta_t = constp.tile([F, 1], f32)
            b2m_t = constp.tile([128, F], f32)
            disT_t = constp.tile([128, CHUNKS], f32)
            disG_t = constp.tile([128, GCHUNKS], f32)
            nc.sync.dma_start(out=W1_t[:], in_=W1_d[:])
            nc.sync.dma_start(out=W2_t[:], in_=W2_d[:])
            nc.sync.dma_start(out=gamma_t[:], in_=gamma_d[:])
            nc.sync.dma_start(out=beta_t[:], in_=beta_d[:])
            nc.sync.dma_start(out=b2m_t[:], in_=b2m_d[:])
            nc.sync.dma_start(out=disT_t[:], in_=disT_d[:])
            nc.sync.dma_start(out=disG_t[:], in_=disG_d[:])
            make_identity(nc, ident_f[:])
            make_identity(nc, ident_b[:])
            iota_i = constp.tile([128, 128], mybir.dt.int32)
            nc.gpsimd.iota(iota_i[:], pattern=[[1, 128]], base=0,
                           channel_multiplier=0)
            for k in range(4):
                nc.vector.tensor_copy(out=iota4[:, k, :], in_=iota_i[:])
            nc.vector.memset(ones_f[:], 1.0)
            nc.vector.memset(ones_b[:], 1.0)

            srcidx_sb = bigp.tile([128, tot // 16], mybir.dt.int16)
            dstloc_sb = bigp.tile([128, ntiles], bf16)
            xsown_sb = bigp.tile([128, CHUNKS, 128], bf16)
            aown_sb = bigp.tile([128, CHUNKS, 128], bf16)
            qtr = (tot // 16) // 4
            for kq in range(4):
                lo = kq * qtr
                hi = (kq + 1) * qtr if kq < 3 else tot // 16
                nc.sync.dma_start(out=srcidx_sb[:, lo:hi],
                                  in_=srcidx_d[:, lo:hi])
            nc.sync.dma_start(out=dstloc_sb[:], in_=dstloc_d[:])
            nc.sync.dma_start(
                out=xsown_sb[:],
                in_=xsown_d.rearrange("(k p) f -> p k f", p=128))

            # ---- shared gather/scatter pass (super-chunk granularity) ----
            # acc_c[f, d] = sum_e table[src_e, f] * onehot[e, d] + own[d, f]
            def layer_pass(table_ap, own_sb, psS, super_stage, hooks=None):
                qn = 0
                for si, sm in enumerate(sup_meta):
                    TS = sm["ntiles"]
                    gb = gbufp.tile([128, max(TS, 1), 128], bf16, tag="gb")
                    base_t = sm["off"] // 128
                    for (b, coff, n) in sm["calls"]:
                        ol = coff - sm["off"]
                        nc.gpsimd.dma_gather(
                            gb[:, ol // 128:(ol + n + 127) // 128, :],
                            table_ap[b * BLK:(b + 1) * BLK, :],
                            srcidx_sb[:, coff // 16:(coff + n) // 16],
                            n, n, F, queue_num=qn,
                            single_packet=(n <= 1024))
                        qn = (qn + 1) % 4
                    accs = [psS.tile([128, F], f32, tag="acc",
                                     name=f"acc{k}")
                            for k in range(SC)]
                    # self-loop seed: acc_c = own_rows_c^T (identity one-hot)
                    for ci, c in enumerate(sm["chunks"]):
                        nc.tensor.matmul(out=accs[ci][:],
                                         lhsT=own_sb[:, c, :],
                                         rhs=ident_b[:],
                                         start=True,
                                         stop=(ci not in sm["last"]))
                    t = 0
                    while t < TS:
                        w = min(4, TS - t)
                        oh = ohp.tile([128, 4, 128], bf16, tag="oh")
                        nc.vector.tensor_tensor(
                            out=oh[:, :w, :],
                            in0=dstloc_sb[:, base_t + t:base_t + t + w]
                                .to_broadcast([128, w, 128]),
                            in1=iota4[:, :w, :], op=OP.is_equal)
                        for j in range(w):
                            ci = sm["chunk_of"][t + j]
                            nc.tensor.matmul(out=accs[ci][:],
                                             lhsT=gb[:, t + j, :],
                                             rhs=oh[:, j, :],
                                             start=False,
                                             stop=(sm["last"][ci] == t + j))
                        t += w
                    super_stage(sm, accs)
                    if hooks and si in hooks:
                        hooks[si]()

            # ================= L1 pass (scoped PSUM pools) =================
            with tc.tile_pool(name="psS1", bufs=SC, space="PSUM") as psS1, \
                 tc.tile_pool(name="psW1", bufs=2, space="PSUM") as psW1, \
                 tc.tile_pool(name="psT1", bufs=2, space="PSUM") as psT1, \
                 tc.tile_pool(name="pss", bufs=1, space="PSUM") as pss:

                # BN stat accumulators (separate banks)
                sum_ps = pss.tile([128, 1], f32, name="sum_ps")
                sq_ps = pss.tile([128, 1], f32, name="sq_ps")

                # ---- L1: acc -> @W1 -> *dis_dst -> stats + transp. ship ----
                def l1_stage(sm, accs):
                    trp = psT1.tile([128, SC, 128], f32, tag="a")
                    for ci, c in enumerate(sm["chunks"]):
                        tc_sb = wp.tile([128, 128], bf16, tag="tc")
                        nc.scalar.activation(tc_sb[:], accs[ci][:], AF.Copy)
                        ps2 = psW1.tile([128, 128], f32, tag="w")
                        nc.tensor.matmul(out=ps2[:], lhsT=tc_sb[:],
                                         rhs=W1_t[:], start=True, stop=True)
                        asb = wp.tile([128, 128], f32, tag="asb")
                        nc.vector.tensor_scalar_mul(out=asb[:], in0=ps2[:],
                                                    scalar1=disT_t[:, c:c + 1])
                        nc.tensor.matmul(out=sum_ps[:], lhsT=asb[:],
                                         rhs=ones_f[:],
                                         start=(c == 0), stop=(c == CHUNKS - 1))
                        sq = wp.tile([128, 128], bf16, tag="sq")
                        nc.scalar.square(sq[:], asb[:])
                        nc.tensor.matmul(out=sq_ps[:], lhsT=sq[:],
                                         rhs=ones_b[:],
                                         start=(c == 0), stop=(c == CHUNKS - 1))
                        nc.tensor.transpose(out=trp[:, ci, :], in_=asb[:],
                                            identity=ident_f[:])
                    tst = wp.tile([128, SC, 128], bf16, tag="tT")
                    nc.scalar.activation(tst[:], trp[:], AF.Copy)
                    r0 = _row_of(sm["chunks"][0])
                    nc.sync.dma_start(
                        out=ag_in[r0:r0 + SC * 128, :]
                            .rearrange("(p k) f -> p k f", p=128),
                        in_=tst[:])

                def ship_first_half():
                    nc.gpsimd.collective_compute(
                        "AllGather", OP.bypass, ins=[ag_in.ap()[0:HROWS, :]],
                        outs=[ag_outA.ap()],
                        replica_groups=[list(range(NCORES))])

                layer_pass(xstab_d, xsown_sb, psS1, l1_stage,
                           hooks={SPLIT // SC - 1: ship_first_half})

                # ---- stats header -> ag_in rows [6400,6528) (rows 0,1) ----
                stats2 = stp.tile([128, 2], f32)
                nc.vector.tensor_copy(out=stats2[:, 0:1], in_=sum_ps[:])
                nc.vector.tensor_copy(out=stats2[:, 1:2], in_=sq_ps[:])
                stpad = stp.tile([128, 128], f32)
                nc.vector.memset(stpad[:], 0.0)
                nc.vector.tensor_copy(out=stpad[:, 0:2], in_=stats2[:])
                trs = psT1.tile([128, SC, 128], f32, tag="a")
                nc.tensor.transpose(out=trs[:, 0, :], in_=stpad[:],
                                    identity=ident_f[:])
                stag = stp.tile([128, 128], bf16)
                nc.scalar.activation(stag[:], trs[:, 0, :], AF.Copy)
                nc.sync.dma_start(out=ag_in[HROWS:HROWS + 128, :], in_=stag[:])

            nc.gpsimd.collective_compute(
                "AllGather", OP.bypass, ins=[ag_in.ap()[HROWS:SEG, :]],
                outs=[ag_outB.ap()],
                replica_groups=[list(range(NCORES))])

            # ================= mid phase (scoped PSUM pools) ===============
            with tc.tile_pool(name="psG", bufs=1, space="PSUM") as psG, \
                 tc.tile_pool(name="psAB", bufs=4, space="PSUM") as psAB:

                # ---- global BN stats from the 8 headers ----
                gst = stp.tile([16, 128], bf16)
                agv = ag_outB.ap().rearrange("(i s) f -> i s f", i=NCORES)
                nc.sync.dma_start(out=gst[0:8, :], in_=agv[:, 0, :])
                nc.sync.dma_start(out=gst[8:16, :], in_=agv[:, 1, :])
                gpad = stp.tile([128, 128], f32)
                nc.vector.memset(gpad[:], 0.0)
                nc.vector.tensor_copy(out=gpad[0:16, :], in_=gst[:])
                gtr = psG.tile([128, 128], f32)
                nc.tensor.transpose(out=gtr[:], in_=gpad[:],
                                    identity=ident_f[:])
                # cols 0..7 = per-core sums, 8..15 = per-core sumsqs
                gred = stp.tile([128, 16], f32)
                nc.vector.tensor_copy(out=gred[:], in_=gtr[:, 0:16])
                nc.vector.tensor_tensor(out=gred[:, 0:4], in0=gred[:, 0:4],
                                        in1=gred[:, 4:8], op=OP.add)
                nc.vector.tensor_tensor(out=gred[:, 8:12], in0=gred[:, 8:12],
                                        in1=gred[:, 12:16], op=OP.add)
                nc.vector.tensor_tensor(out=gred[:, 0:2], in0=gred[:, 0:2],
                                        in1=gred[:, 2:4], op=OP.add)
                nc.vector.tensor_tensor(out=gred[:, 8:10], in0=gred[:, 8:10],
                                        in1=gred[:, 10:12], op=OP.add)
                nc.vector.tensor_tensor(out=gred[:, 0:1], in0=gred[:, 0:1],
                                        in1=gred[:, 1:2], op=OP.add)
                nc.vector.tensor_tensor(out=gred[:, 8:9], in0=gred[:, 8:9],
                                        in1=gred[:, 9:10], op=OP.add)

                mean_t = stp.tile([128, 1], f32)
                ex2_t = stp.tile([128, 1], f32)
                var_t = stp.tile([128, 1], f32)
                sd_t = stp.tile([128, 1], f32)
                rstd_t = stp.tile([128, 1], f32)
                scale_c = stp.tile([128, 1], f32)
                shift_c = stp.tile([128, 1], f32)
                eps_t = stp.tile([128, 1], f32)
                nc.vector.tensor_scalar_mul(out=mean_t[:], in0=gred[:, 0:1],
                                            scalar1=1.0 / N)
                nc.vector.tensor_scalar_mul(out=ex2_t[:], in0=gred[:, 8:9],
                                            scalar1=1.0 / N)
                nc.vector.tensor_tensor(out=var_t[:], in0=mean_t[:],
                                        in1=mean_t[:], op=OP.mult)
                nc.vector.tensor_tensor(out=var_t[:], in0=ex2_t[:],
                                        in1=var_t[:], op=OP.subtract)
                nc.vector.tensor_scalar_max(out=var_t[:], in0=var_t[:],
                                            scalar1=0.0)
                nc.vector.memset(eps_t[:], BN_EPS)
                nc.scalar.activation(sd_t[:], var_t[:], AF.Sqrt, bias=eps_t[:])
                nc.vector.reciprocal(out=rstd_t[:], in_=sd_t[:])
                nc.vector.tensor_tensor(out=scale_c[:], in0=rstd_t[:],
                                        in1=gamma_t[:], op=OP.mult)
                nc.vector.tensor_tensor(out=shift_c[:], in0=mean_t[:],
                                        in1=scale_c[:], op=OP.mult)
                nc.vector.tensor_tensor(out=shift_c[:], in0=beta_t[:],
                                        in1=shift_c[:], op=OP.subtract)

                # activate feature-major groups -> transpose -> *dis -> store
                def act_group(src_ap, dst_sb, dst_c0, c0, w, dis_t, goff):
                    stb = hp.tile([128, w, 128], bf16, tag=f"cb{w}")
                    if w == 2:
                        nc.sync.dma_start(
                            out=stb[:],
                            in_=src_ap.rearrange("(p k) f -> p k f", p=128))
                    else:
                        nc.sync.dma_start(
                            out=stb[:].rearrange("p (s k) f -> p s k f",
                                                 s=w // 2, k=2),
                            in_=src_ap.rearrange("(s p k) f -> p s k f",
                                                 s=w // 2, p=128))
                    h2a = wp.tile([128, w, 128], bf16, tag=f"h2{w}")
                    nc.scalar.activation(h2a[:], stb[:], AF.Relu,
                                         bias=shift_c[:], scale=scale_c[:])
                    trp = psAB.tile([128, 8, 128], bf16, tag="ab")
                    for j in range(w):
                        nc.tensor.transpose(out=trp[:, j, :], in_=h2a[:, j, :],
                                            identity=ident_b[:])
                    for j in range(w):
                        c = c0 + j
                        if c % 3 == 0:
                            nc.scalar.activation(
                                dst_sb[:, dst_c0 + j, :], trp[:, j, :],
                                AF.Copy,
                                scale=dis_t[:, goff + c:goff + c + 1])
                        else:
                            nc.vector.tensor_scalar_mul(
                                out=dst_sb[:, dst_c0 + j, :], in0=trp[:, j, :],
                                scalar1=dis_t[:, goff + c:goff + c + 1])

                RUNS = [(0, SPLIT), (SPLIT, CHUNKS - SPLIT)]

                def half_groups():
                    for base, ln in RUNS:
                        c0 = base
                        while c0 < base + ln:
                            w = min(8, base + ln - c0)
                            yield c0, w
                            c0 += w

                # ---- L2 self rows: a_own*dis from the private ag_in copy ----
                for c0, w in half_groups():
                    r0 = _row_of(c0)
                    act_group(ag_in.ap()[r0:r0 + w * 128, :],
                              aown_sb, c0, c0, w, disT_t, 0)

                # ---- h2s table: relu(bn(agg1))*dis, node-major, all nodes ----
                for i in range(NCORES):
                    for c0, w in half_groups():
                        if c0 < SPLIT:
                            r = i * HROWS + c0 * 128
                            src = ag_outA.ap()[r:r + w * 128, :]
                        else:
                            r = i * SEGB + 128 + (c0 - SPLIT) * 128
                            src = ag_outB.ap()[r:r + w * 128, :]
                        hb2 = hp.tile([128, w, F], bf16, tag=f"hh{w}")
                        act_group(src, hb2, 0, c0, w, disG_t, i * CHUNKS)
                        orow = i * OWN + c0 * 128
                        nc.sync.dma_start(
                            out=h2s[orow:orow + w * 128, :]
                                .rearrange("(k p) f -> p k f", p=128),
                            in_=hb2[:])

            # ================= L2 pass (scoped PSUM pools) =================
            with tc.tile_pool(name="psS2", bufs=SC, space="PSUM") as psS2, \
                 tc.tile_pool(name="psW2", bufs=2, space="PSUM") as psW2:

                # ---- L2: acc -> @W2 -> *dis_dst -> +b2 -> relu -> out ----
                def l2_stage(sm, accs):
                    o1 = wp.tile([128, SC, 128], f32, tag="o1")
                    for ci, c in enumerate(sm["chunks"]):
                        tc_sb = wp.tile([128, 128], bf16, tag="tc")
                        nc.scalar.activation(tc_sb[:], accs[ci][:], AF.Copy)
                        ps2 = psW2.tile([128, 128], f32, tag="w")
                        nc.tensor.matmul(out=ps2[:], lhsT=tc_sb[:],
                                         rhs=W2_t[:], start=True, stop=True)
                        asb = wp.tile([128, 128], f32, tag="asb")
                        nc.vector.tensor_scalar_mul(out=asb[:], in0=ps2[:],
                                                    scalar1=disT_t[:, c:c + 1])
                        nc.vector.tensor_tensor(out=o1[:, ci, :], in0=asb[:],
                                                in1=b2m_t[:], op=OP.add)
                    ot = wp.tile([128, SC, 128], f32, tag="ot")
                    nc.scalar.activation(ot[:], o1[:], AF.Relu)
                    r0 = sm["chunks"][0] * 128
                    nc.sync.dma_start(
                        out=out_d[r0:r0 + SC * 128, :]
                            .rearrange("(k p) f -> p k f", p=128),
                        in_=ot[:])

                layer_pass(h2s.ap(), aown_sb, psS2, l2_stage)

    nc.compile()
    return nc


def kernel(**inputs):
    global LAST_EXEC_NS, LAST_RESULT
    import os
    x = inputs["x"]
    W1 = np.asarray(inputs["W1"], dtype=np.float32)
    W2 = np.asarray(inputs["W2"], dtype=np.float32)
    gamma = np.asarray(inputs["gamma"], dtype=np.float32)
    beta = np.asarray(inputs["beta"], dtype=np.float32)
    b2 = np.asarray(inputs["b2"], dtype=np.float32)
    edge_index = inputs["edge_index"]

    key = (hash(np.asarray(edge_index)[:, ::997].tobytes()),)
    if key not in _cache:
        consts, xs_tab, disG, per_core = _prep(x, edge_index)
        nc = _build(consts)
        _cache[key] = (consts, nc)
    else:
        consts, nc = _cache[key]
        _, xs_tab, disG, per_core = _prep(x, edge_index)

    shared = {
        "xs_tab": xs_tab,
        "disG": disG,
        "W1b": W1.astype(BF16), "W2b": W2.astype(BF16),
        "gamma_c": gamma.reshape(F, 1).copy(),
        "beta_c": beta.reshape(F, 1).copy(),
        "b2_mat": np.ascontiguousarray(np.broadcast_to(b2.reshape(1, F),
                                                       (128, F))),
    }
    in_maps = []
    for i in range(NCORES):
        m = dict(shared)
        m.update(per_core[i])
        in_maps.append(m)

    trace = bool(os.environ.get("BASS_GCN_TRACE"))
    res = run_bass_kernel_spmd(nc, in_maps, list(range(NCORES)), trace=trace)
    LAST_EXEC_NS = res.exec_time_ns
    LAST_RESULT = res

    out = np.concatenate([res.results[i]["out"] for i in range(NCORES)], axis=0)
    return np.ascontiguousarray(out[:N]).astype(np.float32)


# revision 28
# speedup vs baseline: 1.0854x; 1.0377x over previous
"""2-layer GCN (GCNConv -> BatchNorm(train) -> ReLU -> GCNConv -> ReLU) on 8 TRN2
NeuronCores, SPMD (one NEFF on all cores).

v12 design (evolved from v3 via NTFF profiles; 2794us -> 1862us):
  - W applied AFTER aggregation (matmul commutes with the scatter-sum):
    L1 gathers raw xs = x*dis rows from a host-shipped node-major table,
    so the per-core h1 table build (52MB HBM + 800 matmuls) is gone.
  - Self-loops are synthetic identity-matmul tiles (lhsT=own rows,
    rhs=identity) seeding each chunk's PSUM accumulator - no gather
    descriptors, no separate self-term passes.
  - dis_src folded into gather-table rows (xs host-side; h2s rows scaled
    during the table build), dis_dst applied per-chunk post-matmul: the
    one-hot is a bare is_eq for BOTH layers (v3 spent ~290us/layer on the
    dissrc multiply, and tensor_tensor with a broadcast operand runs in
    1x DVE mode anyway).
  - Gather calls ~1920 idxs with single_packet=False: SWDGE packets cap at
    ~64 descriptors, so single_packet=True calls beyond 1024 idxs wedge
    the queue (hard device hang); multi-packet big calls amortize the
    ~1us/call fixed cost (was the v3 pacer: GpSimd 65% busy, all in
    per-call SWDGE overhead at 371 calls/layer of <=1024).
  - AllGather split: chunks 0-(SPLIT-1) ship mid-L1 (hidden under the
    gather pass - the mesh AG waits ~70us/MB), header+rest after L1.
    Downstream gates on the LAST collective, so more splits don't help.
  - BN stats ride the second AG's header; L2 self rows are rebuilt from
    the private ag_in copy (no per-core control flow anywhere).
  - AG payload rows are (p k)-interleaved per super so the table-build
    readers see 512B-contiguous runs per partition (halves descriptor
    count vs strict row-major; 256B descs pay a 2x DMA penalty).
  - Mid-phase table build runs width-8 chunk groups with bf16
    activations and bf16 PE-array transposes (fewer, wider chain stages;
    fits PSUM/SBUF where the f32 variant overflowed).
  - PSUM pools are bank-granular (8 banks): scoped per phase.

Sharding: nodes padded 100000 -> 102400 = 8*12800, core i owns rows
[i*12800,(i+1)*12800); edges partitioned by dst owner; weights replicated.
"""
import numpy as np
import ml_dtypes

import concourse.bass as bass
import concourse.mybir as mybir
import concourse.tile as tile
from concourse import bacc
from concourse.bass_utils import run_bass_kernel_spmd
from concourse.masks import make_identity

N = 100000
F = 128
NCORES = 8
NPAD = 102400
OWN = NPAD // NCORES          # 12800
CHUNKS = OWN // 128           # 100
SPLIT = 70                    # chunks shipped in the first AG
HROWS = SPLIT * 128           # 8960
GCHUNKS = NPAD // NCORES * NCORES // 128  # 800
NBLK = 4
BLK = NPAD // NBLK            # 25600 (< 32768, int16-addressable)
SEG = OWN + 128               # 12928 rows: c0-69 | header | c70-99
BN_EPS = 1e-5
SC = 2                        # dst chunks per super-chunk
QCAP = 1920                   # max idxs per gather call (121 ring descs;
                              # a call must stay under the 128-desc SWDGE
                              # inflight window or the queue wedges)
BF16 = ml_dtypes.bfloat16

LAST_EXEC_NS = None
LAST_RESULT = None
_cache = {}


def _row_of(c):
    """ag_in row of chunk c's first row (header lives at [HROWS, HROWS+128))."""
    return c * 128 if c < SPLIT else HROWS + 128 + (c - SPLIT) * 128


def _prep(x, edge_index):
    src = np.asarray(edge_index[0]).astype(np.int64)
    dst = np.asarray(edge_index[1]).astype(np.int64)

    deg = np.bincount(dst, minlength=N).astype(np.float32) + 1.0
    dis = np.zeros(NPAD, dtype=np.float32)
    dis[:N] = 1.0 / np.sqrt(deg)

    xs = np.zeros((NPAD, F), dtype=np.float32)
    xs[:N] = np.asarray(x, dtype=np.float32) * dis[:N, None]
    xs_tab = np.ascontiguousarray(xs.astype(BF16))         # [NPAD, F] bf16

    owner = dst // OWN
    chunk = (dst % OWN) // 128
    blk = src // BLK
    cell = ((owner * CHUNKS + chunk) * NBLK + blk).astype(np.int64)
    order = np.lexsort((src, cell))      # ascending src within each cell
    src_s = src[order]
    dst_s = dst[order]

    counts = np.zeros((NCORES, CHUNKS, NBLK), np.int64)
    np.add.at(counts, (owner, chunk, blk), 1)
    C = counts.max(axis=0)
    C = ((C + 127) // 128) * 128         # zero cells stay zero

    starts = np.zeros(NCORES * CHUNKS * NBLK + 1, dtype=np.int64)
    starts[1:] = np.cumsum(counts.reshape(-1))

    # super-chunk slot layout: for each super s: for each block b: the SC
    # cells (c, b) back to back.  Gather call = one (s, b) segment, split
    # to <=QCAP idxs (balanced so no tiny remainder call).
    nsup = CHUNKS // SC
    slot_pos = {}
    sup_meta = []
    off = 0
    for s in range(nsup):
        chs = list(range(s * SC, (s + 1) * SC))
        sup_off = off
        seg_calls = []
        for b in range(NBLK):
            call_off = off
            for c in chs:
                slot_pos[(c, b)] = off
                off += int(C[c, b])
            seg_n = off - call_off
            if seg_n:
                k = -(-seg_n // QCAP)            # calls for this segment
                per = ((seg_n // k) // 128) * 128
                sub = 0
                for ki in range(k):
                    n = per if ki < k - 1 else seg_n - per * (k - 1)
                    assert 0 < n <= 2032, n   # 128-desc inflight window
                    seg_calls.append((b, call_off + sub, n))
                    sub += n
        chunk_of = []
        for b in range(NBLK):
            for ci, c in enumerate(chs):
                chunk_of.extend([ci] * (int(C[c, b]) // 128))
        last = {}
        for t, ci in enumerate(chunk_of):
            last[ci] = t
        sup_meta.append({"off": sup_off, "ntiles": len(chunk_of),
                         "chunk_of": chunk_of, "last": last,
                         "calls": seg_calls, "chunks": chs})
    tot = off
    ntiles = tot // 128

    per_core = []
    for i in range(NCORES):
        srcidx = np.zeros(tot, dtype=np.int16)                # pads hit row 0
        dstloc = np.full(tot, -1.0, dtype=np.float32)         # pads no column
        for c in range(CHUNKS):
            for b in range(NBLK):
                k = (i * CHUNKS + c) * NBLK + b
                m = int(counts[i, c, b])
                if m:
                    o = slot_pos[(c, b)]
                    sl = slice(starts[k], starts[k] + m)
                    srcidx[o:o + m] = (src_s[sl] - b * BLK).astype(np.int16)
                    dstloc[o:o + m] = (dst_s[sl] % 128).astype(np.float32)
        iw = srcidx.reshape(tot // 16, 16).T                  # [16, tot/16]
        srcidx_w = np.ascontiguousarray(np.tile(iw, (8, 1)))  # [128, tot/16]
        dstloc_t = np.ascontiguousarray(
            dstloc.reshape(ntiles, 128).T.astype(BF16))
        disT = np.ascontiguousarray(
            dis[i * OWN:(i + 1) * OWN].reshape(CHUNKS, 128).T)
        xs_own = np.ascontiguousarray(xs_tab[i * OWN:(i + 1) * OWN])
        per_core.append({"srcidx": srcidx_w, "dstloc": dstloc_t,
                         "disT": disT, "xs_own": xs_own})

    disG = np.ascontiguousarray(dis.reshape(GCHUNKS, 128).T)  # [128, 800]

    consts = {"tot": tot, "ntiles": ntiles, "sup_meta": sup_meta}
    return consts, xs_tab, disG, per_core


def _build(consts):
    tot = consts["tot"]
    ntiles = consts["ntiles"]
    sup_meta = consts["sup_meta"]

    f32 = mybir.dt.float32
    bf16 = mybir.dt.bfloat16
    AF = mybir.ActivationFunctionType
    OP = mybir.AluOpType
    nc = bacc.Bacc("TRN2", target_bir_lowering=False, debug=False,
                   num_devices=NCORES, num_swdge_queues=4,
                   dynamic_dma_scratch_size=32768)

    xstab_d = nc.dram_tensor("xs_tab", [NPAD, F], bf16, kind="ExternalInput").ap()
    xsown_d = nc.dram_tensor("xs_own", [OWN, F], bf16, kind="ExternalInput").ap()
    W1_d = nc.dram_tensor("W1b", [F, F], bf16, kind="ExternalInput").ap()
    W2_d = nc.dram_tensor("W2b", [F, F], bf16, kind="ExternalInput").ap()
    gamma_d = nc.dram_tensor("gamma_c", [F, 1], f32, kind="ExternalInput").ap()
    beta_d = nc.dram_tensor("beta_c", [F, 1], f32, kind="ExternalInput").ap()
    b2m_d = nc.dram_tensor("b2_mat", [128, F], f32, kind="ExternalInput").ap()
    disT_d = nc.dram_tensor("disT", [128, CHUNKS], f32, kind="ExternalInput").ap()
    disG_d = nc.dram_tensor("disG", [128, GCHUNKS], f32, kind="ExternalInput").ap()
    srcidx_d = nc.dram_tensor("srcidx", [128, tot // 16], mybir.dt.int16,
                              kind="ExternalInput").ap()
    dstloc_d = nc.dram_tensor("dstloc", [128, ntiles], bf16,
                              kind="ExternalInput").ap()
    out_d = nc.dram_tensor("out", [OWN, F], f32, kind="ExternalOutput").ap()

    h2s = nc.dram_tensor("h2s_tab", [NPAD, F], bf16)
    ag_in = nc.dram_tensor("ag_in", [SEG, F], bf16)
    ag_outA = nc.dram_tensor("ag_outA", [NCORES * HROWS, F], bf16,
                             addr_space="Shared")
    SEGB = SEG - HROWS            # 3968 rows: header + chunks 70-99
    ag_outB = nc.dram_tensor("ag_outB", [NCORES * SEGB, F], bf16,
                             addr_space="Shared")

    with tile.TileContext(nc) as tc:
        with tc.tile_pool(name="const", bufs=1) as constp, \
             tc.tile_pool(name="big", bufs=1) as bigp, \
             tc.tile_pool(name="h", bufs=3) as hp, \
             tc.tile_pool(name="gbuf", bufs=4) as gbufp, \
             tc.tile_pool(name="oh", bufs=6) as ohp, \
             tc.tile_pool(name="wk", bufs=4) as wp, \
             tc.tile_pool(name="st", bufs=1) as stp:

            # ---- constants ----
            W1_t = constp.tile([F, F], bf16)
            W2_t = constp.tile([F, F], bf16)
            ident_f = constp.tile([128, 128], f32)
            ident_b = constp.tile([128, 128], bf16)
            iota4 = constp.tile([128, 4, 128], bf16)
            ones_f = constp.tile([128, 1], f32)
            ones_b = constp.tile([128, 1], bf16)
            gamma_t = constp.tile([F, 1], f32)
            beta_t = constp.tile([F, 1], f32)
            b2m_t = constp.tile([128, F], f32)
            disT_t = constp.tile([128, CHUNKS], f32)
            disG_t = constp.tile([128, GCHUNKS], f32)
            nc.sync.dma_start(out=W1_t[:], in_=W1_d[:])
            nc.sync.dma_start(out=W2_t[:], in_=W2_d[:])
            nc.sync.dma_start(out=gamma_t[:], in_=gamma_d[:])
            nc.sync.dma_start(out=beta_t[:], in_=beta_d[:])
            nc.sync.dma_start(out=b2m_t[:], in_=b2m_d[:])
            nc.sync.dma_start(out=disT_t[:], in_=disT_d[:])
            nc.sync.dma_start(out=disG_t[:], in_=disG_d[:])
            make_identity(nc, ident_f[:])
            make_identity(nc, ident_b[:])
            iota_i = constp.tile([128, 128], mybir.dt.int32)
            nc.gpsimd.iota(iota_i[:], pattern=[[1, 128]], base=0,
                           channel_multiplier=0)
            for k in range(4):
                nc.vector.tensor_copy(out=iota4[:, k, :], in_=iota_i[:])
            nc.vector.memset(ones_f[:], 1.0)
            nc.vector.memset(ones_b[:], 1.0)

            srcidx_sb = bigp.tile([128, tot // 16], mybir.dt.int16)
            dstloc_sb = bigp.tile([128, ntiles], bf16)
            xsown_sb = bigp.tile([128, CHUNKS, 128], bf16)
            aown_sb = bigp.tile([128, CHUNKS, 128], bf16)
            qtr = (tot // 16) // 4
            for kq in range(4):
                lo = kq * qtr
                hi = (kq + 1) * qtr if kq < 3 else tot // 16
                nc.sync.dma_start(out=srcidx_sb[:, lo:hi],
                                  in_=srcidx_d[:, lo:hi])
            nc.sync.dma_start(out=dstloc_sb[:], in_=dstloc_d[:])
            nc.sync.dma_start(
                out=xsown_sb[:],
                in_=xsown_d.rearrange("(k p) f -> p k f", p=128))

            # ---- shared gather/scatter pass (super-chunk granularity) ----
            # acc_c[f, d] = sum_e table[src_e, f] * onehot[e, d] + own[d, f]
            def layer_pass(table_ap, own_sb, psS, super_stage, hooks=None):
                qn = 0
                for si, sm in enumerate(sup_meta):
                    TS = sm["ntiles"]
                    gb = gbufp.tile([128, max(TS, 1), 128], bf16, tag="gb")
                    base_t = sm["off"] // 128
                    for (b, coff, n) in sm["calls"]:
                        ol = coff - sm["off"]
                        nc.gpsimd.dma_gather(
                            gb[:, ol // 128:(ol + n + 127) // 128, :],
                            table_ap[b * BLK:(b + 1) * BLK, :],
                            srcidx_sb[:, coff // 16:(coff + n) // 16],
                            n, n, F, queue_num=qn,
                            single_packet=(n <= 1024))
                        qn = (qn + 1) % 4
                    accs = [psS.tile([128, F], f32, tag="acc",
                                     name=f"acc{k}")
                            for k in range(SC)]
                    # self-loop seed: acc_c = own_rows_c^T (identity one-hot)
                    for ci, c in enumerate(sm["chunks"]):
                        nc.tensor.matmul(out=accs[ci][:],
                                         lhsT=own_sb[:, c, :],
                                         rhs=ident_b[:],
                                         start=True,
                                         stop=(ci not in sm["last"]))
                    t = 0
                    while t < TS:
                        w = min(4, TS - t)
                        oh = ohp.tile([128, 4, 128], bf16, tag="oh")
                        nc.vector.tensor_tensor(
                            out=oh[:, :w, :],
                            in0=dstloc_sb[:, base_t + t:base_t + t + w]
                                .to_broadcast([128, w, 128]),
                            in1=iota4[:, :w, :], op=OP.is_equal)
                        for j in range(w):
                            ci = sm["chunk_of"][t + j]
                            nc.tensor.matmul(out=accs[ci][:],
                                             lhsT=gb[:, t + j, :],
                                             rhs=oh[:, j, :],
                                             start=False,
                                             stop=(sm["last"][ci] == t + j))
                        t += w
                    super_stage(sm, accs)
                    if hooks and si in hooks:
                        hooks[si]()

            # ================= L1 pass (scoped PSUM pools) =================
            with tc.tile_pool(name="psS1", bufs=SC, space="PSUM") as psS1, \
                 tc.tile_pool(name="psW1", bufs=2, space="PSUM") as psW1, \
                 tc.tile_pool(name="psT1", bufs=2, space="PSUM") as psT1, \
                 tc.tile_pool(name="pss", bufs=1, space="PSUM") as pss:

                # BN stat accumulators (separate banks)
                sum_ps = pss.tile([128, 1], f32, name="sum_ps")
                sq_ps = pss.tile([128, 1], f32, name="sq_ps")

                # ---- L1: acc -> @W1 -> *dis_dst -> stats + transp. ship ----
                def l1_stage(sm, accs):
                    trp = psT1.tile([128, SC, 128], f32, tag="a")
                    for ci, c in enumerate(sm["chunks"]):
                        tc_sb = wp.tile([128, 128], bf16, tag="tc")
                        nc.scalar.activation(tc_sb[:], accs[ci][:], AF.Copy)
                        ps2 = psW1.tile([128, 128], f32, tag="w")
                        nc.tensor.matmul(out=ps2[:], lhsT=tc_sb[:],
                                         rhs=W1_t[:], start=True, stop=True)
                        asb = wp.tile([128, 128], f32, tag="asb")
                        nc.vector.tensor_scalar_mul(out=asb[:], in0=ps2[:],
                                                    scalar1=disT_t[:, c:c + 1])
                        nc.tensor.matmul(out=sum_ps[:], lhsT=asb[:],
                                         rhs=ones_f[:],
                                         start=(c == 0), stop=(c == CHUNKS - 1))
                        sq = wp.tile([128, 128], bf16, tag="sq")
                        nc.scalar.square(sq[:], asb[:])
                        nc.tensor.matmul(out=sq_ps[:], lhsT=sq[:],
                                         rhs=ones_b[:],
                                         start=(c == 0), stop=(c == CHUNKS - 1))
                        nc.tensor.transpose(out=trp[:, ci, :], in_=asb[:],
                                            identity=ident_f[:])
                    tst = wp.tile([128, SC, 128], bf16, tag="tT")
                    nc.scalar.activation(tst[:], trp[:], AF.Copy)
                    r0 = _row_of(sm["chunks"][0])
                    nc.sync.dma_start(
                        out=ag_in[r0:r0 + SC * 128, :]
                            .rearrange("(p k) f -> p k f", p=128),
                        in_=tst[:])

                def ship_first_half():
                    nc.gpsimd.collective_compute(
                        "AllGather", OP.bypass, ins=[ag_in.ap()[0:HROWS, :]],
                        outs=[ag_outA.ap()],
                        replica_groups=[list(range(NCORES))])

                layer_pass(xstab_d, xsown_sb, psS1, l1_stage,
                           hooks={SPLIT // SC - 1: ship_first_half})

                # ---- stats header -> ag_in rows [6400,6528) (rows 0,1) ----
                stats2 = stp.tile([128, 2], f32)
                nc.vector.tensor_copy(out=stats2[:, 0:1], in_=sum_ps[:])
                nc.vector.tensor_copy(out=stats2[:, 1:2], in_=sq_ps[:])
                stpad = stp.tile([128, 128], f32)
                nc.vector.memset(stpad[:], 0.0)
                nc.vector.tensor_copy(out=stpad[:, 0:2], in_=stats2[:])
                trs = psT1.tile([128, SC, 128], f32, tag="a")
                nc.tensor.transpose(out=trs[:, 0, :], in_=stpad[:],
                                    identity=ident_f[:])
                stag = stp.tile([128, 128], bf16)
                nc.scalar.activation(stag[:], trs[:, 0, :], AF.Copy)
                nc.sync.dma_start(out=ag_in[HROWS:HROWS + 128, :], in_=stag[:])

            nc.gpsimd.collective_compute(
                "AllGather", OP.bypass, ins=[ag_in.ap()[HROWS:SEG, :]],
                outs=[ag_outB.ap()],
                replica_groups=[list(range(NCORES))])

            # ================= mid phase (scoped PSUM pools) ===============
            with tc.tile_pool(name="psG", bufs=1, space="PSUM") as psG, \
                 tc.tile_pool(name="psAB", bufs=4, space="PSUM") as psAB:

                # ---- global BN stats from the 8 headers ----
                gst = stp.tile([16, 128], bf16)
                agv = ag_outB.ap().rearrange("(i s) f -> i s f", i=NCORES)
                nc.sync.dma_start(out=gst[0:8, :], in_=agv[:, 0, :])
                nc.sync.dma_start(out=gst[8:16, :], in_=agv[:, 1, :])
                gpad = stp.tile([128, 128], f32)
                nc.vector.memset(gpad[:], 0.0)
                nc.vector.tensor_copy(out=gpad[0:16, :], in_=gst[:])
                gtr = psG.tile([128, 128], f32)
                nc.tensor.transpose(out=gtr[:], in_=gpad[:],
                                    identity=ident_f[:])
                # cols 0..7 = per-core sums, 8..15 = per-core sumsqs
                gred = stp.tile([128, 16], f32)
                nc.vector.tensor_copy(out=gred[:], in_=gtr[:, 0:16])
                nc.vector.tensor_tensor(out=gred[:, 0:4], in0=gred[:, 0:4],
                                        in1=gred[:, 4:8], op=OP.add)
                nc.vector.tensor_tensor(out=gred[:, 8:12], in0=gred[:, 8:12],
                                        in1=gred[:, 12:16], op=OP.add)
                nc.vector.tensor_tensor(out=gred[:, 0:2], in0=gred[:, 0:2],
                                        in1=gred[:, 2:4], op=OP.add)
                nc.vector.tensor_tensor(out=gred[:, 8:10], in0=gred[:, 8:10],
                                        in1=gred[:, 10:12], op=OP.add)
                nc.vector.tensor_tensor(out=gred[:, 0:1], in0=gred[:, 0:1],
                                        in1=gred[:, 1:2], op=OP.add)
                nc.vector.tensor_tensor(out=gred[:, 8:9], in0=gred[:, 8:9],
                                        in1=gred[:, 9:10], op=OP.add)

                mean_t = stp.tile([128, 1], f32)
                ex2_t = stp.tile([128, 1], f32)
                var_t = stp.tile([128, 1], f32)
                sd_t = stp.tile([128, 1], f32)
                rstd_t = stp.tile([128, 1], f32)
                scale_c = stp.tile([128, 1], f32)
                shift_c = stp.tile([128, 1], f32)
                eps_t = stp.tile([128, 1], f32)
                nc.vector.tensor_scalar_mul(out=mean_t[:], in0=gred[:, 0:1],
                                            scalar1=1.0 / N)
                nc.vector.tensor_scalar_mul(out=ex2_t[:], in0=gred[:, 8:9],
                                            scalar1=1.0 / N)
                nc.vector.tensor_tensor(out=var_t[:], in0=mean_t[:],
                                        in1=mean_t[:], op=OP.mult)
                nc.vector.tensor_tensor(out=var_t[:], in0=ex2_t[:],
                                        in1=var_t[:], op=OP.subtract)
                nc.vector.tensor_scalar_max(out=var_t[:], in0=var_t[:],
                                            scalar1=0.0)
                nc.vector.memset(eps_t[:], BN_EPS)
                nc.scalar.activation(sd_t[:], var_t[:], AF.Sqrt, bias=eps_t[:])
                nc.vector.reciprocal(out=rstd_t[:], in_=sd_t[:])
                nc.vector.tensor_tensor(out=scale_c[:], in0=rstd_t[:],
                                        in1=gamma_t[:], op=OP.mult)
                nc.vector.tensor_tensor(out=shift_c[:], in0=mean_t[:],
                                        in1=scale_c[:], op=OP.mult)
                nc.vector.tensor_tensor(out=shift_c[:], in0=beta_t[:],
                                        in1=shift_c[:], op=OP.subtract)

                # activate feature-major groups -> transpose -> *dis -> store
                def act_group(src_ap, dst_sb, dst_c0, c0, w, dis_t, goff):
                    stb = hp.tile([128, w, 128], bf16, tag=f"cb{w}")
                    if w == 2:
                        nc.sync.dma_start(
                            out=stb[:],
                            in_=src_ap.rearrange("(p k) f -> p k f", p=128))
                    else:
                        nc.sync.dma_start(
                            out=stb[:].rearrange("p (s k) f -> p s k f",
                                                 s=w // 2, k=2),
                            in_=src_ap.rearrange("(s p k) f -> p s k f",
                                                 s=w // 2, p=128))
                    h2a = wp.tile([128, w, 128], bf16, tag=f"h2{w}")
                    nc.scalar.activation(h2a[:], stb[:], AF.Relu,
                                         bias=shift_c[:], scale=scale_c[:])
                    trp = psAB.tile([128, 8, 128], bf16, tag="ab")
                    for j in range(w):
                        nc.tensor.transpose(out=trp[:, j, :], in_=h2a[:, j, :],
                                            identity=ident_b[:])
                    for j in range(w):
                        c = c0 + j
                        if c % 3 == 0:
                            nc.scalar.activation(
                                dst_sb[:, dst_c0 + j, :], trp[:, j, :],
                                AF.Copy,
                                scale=dis_t[:, goff + c:goff + c + 1])
                        else:
                            nc.vector.tensor_scalar_mul(
                                out=dst_sb[:, dst_c0 + j, :], in0=trp[:, j, :],
                                scalar1=dis_t[:, goff + c:goff + c + 1])

                RUNS = [(0, SPLIT), (SPLIT, CHUNKS - SPLIT)]

                def half_groups():
                    for base, ln in RUNS:
                        c0 = base
                        while c0 < base + ln:
                            w = min(8, base + ln - c0)
                            yield c0, w
                            c0 += w

                # ---- L2 self rows: a_own*dis from the private ag_in copy ----
                for c0, w in half_groups():
                    r0 = _row_of(c0)
                    act_group(ag_in.ap()[r0:r0 + w * 128, :],
                              aown_sb, c0, c0, w, disT_t, 0)

                # ---- h2s table: relu(bn(agg1))*dis, node-major, all nodes ----
                for i in range(NCORES):
                    for c0, w in half_groups():
                        if c0 < SPLIT:
                            r = i * HROWS + c0 * 128
                            src = ag_outA.ap()[r:r + w * 128, :]
                        else:
                            r = i * SEGB + 128 + (c0 - SPLIT) * 128
                            src = ag_outB.ap()[r:r + w * 128, :]
                        hb2 = hp.tile([128, w, F], bf16, tag=f"hh{w}")
                        act_group(src, hb2, 0, c0, w, disG_t, i * CHUNKS)
                        orow = i * OWN + c0 * 128
                        nc.sync.dma_start(
                            out=h2s[orow:orow + w * 128, :]
                                .rearrange("(k p) f -> p k f", p=128),
                            in_=hb2[:])

            # ================= L2 pass (scoped PSUM pools) =================
            with tc.tile_pool(name="psS2", bufs=SC, space="PSUM") as psS2, \
                 tc.tile_pool(name="psW2", bufs=2, space="PSUM") as psW2:

                # ---- L2: acc -> @W2 -> *dis_dst -> +b2 -> relu -> out ----
                def l2_stage(sm, accs):
                    o1 = wp.tile([128, SC, 128], f32, tag="o1")
                    for ci, c in enumerate(sm["chunks"]):
                        tc_sb = wp.tile([128, 128], bf16, tag="tc")
                        nc.scalar.activation(tc_sb[:], accs[ci][:], AF.Copy)
                        ps2 = psW2.tile([128, 128], f32, tag="w")
                        nc.tensor.matmul(out=ps2[:], lhsT=tc_sb[:],
                                         rhs=W2_t[:], start=True, stop=True)
                        asb = wp.tile([128, 128], f32, tag="asb")
                        nc.vector.tensor_scalar_mul(out=asb[:], in0=ps2[:],
                                                    scalar1=disT_t[:, c:c + 1])
                        nc.vector.tensor_tensor(out=o1[:, ci, :], in0=asb[:],
                                                in1=b2m_t[:], op=OP.add)
                    ot = wp.tile([128, SC, 128], f32, tag="ot")
                    nc.scalar.activation(ot[:], o1[:], AF.Relu)
                    r0 = sm["chunks"][0] * 128
                    nc.sync.dma_start(
                        out=out_d[r0:r0 + SC * 128, :]
                            .rearrange("(k p) f -> p k f", p=128),
                        in_=ot[:])

                layer_pass(h2s.ap(), aown_sb, psS2, l2_stage)

    nc.compile()
    return nc


def kernel(**inputs):
    global LAST_EXEC_NS, LAST_RESULT
    import os
    x = inputs["x"]
    W1 = np.asarray(inputs["W1"], dtype=np.float32)
    W2 = np.asarray(inputs["W2"], dtype=np.float32)
    gamma = np.asarray(inputs["gamma"], dtype=np.float32)
    beta = np.asarray(inputs["beta"], dtype=np.float32)
    b2 = np.asarray(inputs["b2"], dtype=np.float32)
    edge_index = inputs["edge_index"]

    key = (hash(np.asarray(edge_index)[:, ::997].tobytes()),)
    if key not in _cache:
        consts, xs_tab, disG, per_core = _prep(x, edge_index)
        nc = _build(consts)
        _cache[key] = (consts, nc)
    else:
        consts, nc = _cache[key]
        _, xs_tab, disG, per_core = _prep(x, edge_index)

    shared = {
        "xs_tab": xs_tab,
        "disG": disG,
        "W1b": W1.astype(BF16), "W2b": W2.astype(BF16),
        "gamma_c": gamma.reshape(F, 1).copy(),
        "beta_c": beta.reshape(F, 1).copy(),
        "b2_mat": np.ascontiguousarray(np.broadcast_to(b2.reshape(1, F),
                                                       (128, F))),
    }
    in_maps = []
    for i in range(NCORES):
        m = dict(shared)
        m.update(per_core[i])
        in_maps.append(m)

    trace = bool(os.environ.get("BASS_GCN_TRACE"))
    res = run_bass_kernel_spmd(nc, in_maps, list(range(NCORES)), trace=trace)
    LAST_EXEC_NS = res.exec_time_ns
    LAST_RESULT = res

    out = np.concatenate([res.results[i]["out"] for i in range(NCORES)], axis=0)
    return np.ascontiguousarray(out[:N]).astype(np.float32)
